# revision 1
# baseline (speedup 1.0000x reference)
"""BERT self-attention (BS=4, SEQ=2048, HID=768, NH=12) on 8 NeuronCores.

Sharding: core c -> batch b = c//2, head-group g = c%2 (6 heads each).
Per core the Bass kernel computes, for its batch element and 6 heads:
  Q^T/K^T = (Wh @ X^T + b)   in [d, q] layout  (d on partitions)
  V       = X @ Wv^T + bv    in [k, d] layout, rows scaled by mask m[k]
  S^T     = K^T.T-free matmul -> [k_block, q] scores in PSUM
  P^T     = exp(S^T / 8)     (ACT, PSUM->SBUF; mask folded into V)
  ctx^T   = V'.T @ P^T accumulated over k blocks, with a 65th row = mask
            column giving the softmax denominator.
  out     = ctx^T[0:64] * broadcast(1/denom)  -> [64, q] per head
Host does input transposes (free), sharding, and the final [d,q]->[q,d]
untranspose + concat.

Biases are folded in via an appended ones-row on X^T (contraction 769).
"""

import numpy as np

import concourse.bass as bass
import concourse.tile as tile
from concourse import bacc
from concourse import mybir
from concourse.bass_utils import run_bass_kernel_spmd

F32 = mybir.dt.float32
F32R = mybir.dt.float32r
F16 = mybir.dt.float16
DT_MM = F16          # dtype for matmul operands (A/B: F16 vs F32R)
DT_NP = np.float16   # matching numpy dtype for host-side input prep

BS, SEQ, HID, NH, HD = 4, 2048, 768, 12, 64
NCORES = 8
HPC = 6          # heads per core
FCH = 6          # 128-row chunks of the 768 contraction dim
DSH = HPC * HD   # 384 output features per core


def _body(tc, xt_d, wq_d, wk_d, wv_d, mt_d, ot_d):
    nc = tc.nc
    Exp = mybir.ActivationFunctionType.Exp

    with tc.tile_pool(name="persist", bufs=1) as persist:
        # Warm the exp table set ASAP (overlaps the input DMAs).
        dummy = persist.tile([1, 1], F32, tag="dummy")
        nc.vector.memset(dummy, 0.0)
        nc.scalar.activation(out=dummy, in_=dummy, func=Exp)

        mtile = persist.tile([128, 16], DT_MM, tag="mtile")
        nc.sync.dma_start(out=mtile, in_=mt_d[:, :])
        mtf = persist.tile([128, 16], F32, tag="mtf")
        nc.vector.tensor_copy(out=mtf, in_=mtile)
        qt = [persist.tile([128, SEQ], DT_MM, tag=f"qt{j}", name=f"qt{j}") for j in range(3)]
        kt = [persist.tile([128, SEQ], DT_MM, tag=f"kt{j}", name=f"kt{j}") for j in range(3)]
        vt = persist.tile([128, 16, DSH], DT_MM, tag="vt")

        # ---------------- Phase 1: QKV projections ----------------
        with tc.tile_pool(name="xw", bufs=1) as xw, \
             tc.tile_pool(name="qkp", bufs=3, space="PSUM") as qkp, \
             tc.tile_pool(name="vp", bufs=2, space="PSUM") as vp:
            xts = []
            for f in range(FCH):
                t = xw.tile([128, SEQ], DT_MM, tag=f"x{f}")
                nc.sync.dma_start(out=t, in_=xt_d[f * 128:(f + 1) * 128, :])
                xts.append(t)
            xt1 = persist.tile([1, SEQ], DT_MM, tag="x6")
            nc.sync.dma_start(out=xt1, in_=xt_d[768:769, :])

            wmap = {}
            for dram, nm in ((wq_d, "q"), (wk_d, "k"), (wv_d, "v")):
                lst = []
                for f in range(FCH):
                    t = xw.tile([128, DSH], DT_MM, tag=f"w{nm}{f}")
                    nc.sync.dma_start(out=t, in_=dram[f * 128:(f + 1) * 128, :])
                    lst.append(t)
                b = xw.tile([1, DSH], DT_MM, tag=f"w{nm}b")
                nc.sync.dma_start(out=b, in_=dram[768:769, :])
                lst.append(b)
                wmap[nm] = lst

            # Q^T, K^T: [384, 2048] as 3 tiles of [128, 2048]
            for nm, dst in (("q", qt), ("k", kt)):
                wt = wmap[nm]
                for j in range(3):
                    js = slice(j * 128, (j + 1) * 128)
                    for qc in range(4):
                        qs = slice(qc * 512, (qc + 1) * 512)
                        ps = qkp.tile([128, 512], F32, tag="qk")
                        for f in range(FCH):
                            nc.tensor.matmul(ps, lhsT=wt[f][:, js],
                                             rhs=xts[f][:, qs],
                                             start=(f == 0), stop=False)
                        nc.tensor.matmul(ps, lhsT=wt[6][:, js],
                                         rhs=xt1[:, qs],
                                         start=False, stop=True)
                        nc.scalar.copy(out=dst[j][:, qs], in_=ps)

            # V: [2048, 384] as 16 k-blocks, mask-scaled rows
            wt = wmap["v"]
            for kb in range(16):
                ks = slice(kb * 128, (kb + 1) * 128)
                ps = vp.tile([128, DSH], F32, tag="v")
                for f in range(FCH):
                    nc.tensor.matmul(ps, lhsT=xts[f][:, ks], rhs=wt[f],
                                     start=(f == 0), stop=False)
                nc.tensor.matmul(ps, lhsT=xt1[:, ks], rhs=wt[6],
                                 start=False, stop=True)
                nc.vector.tensor_scalar_mul(
                    out=vt[:, kb, :], in0=ps,
                    scalar1=mtf[:, kb:kb + 1])

        # ---------------- Phase 2: attention ----------------
        # PSUM: scores 2x[128,1024] (4 banks) + ctx pair [128,1024] (2) +
        # denoms [97,1024] (2) = 8 banks.
        with tc.tile_pool(name="sp", bufs=2, space="PSUM") as sp, \
             tc.tile_pool(name="cp", bufs=1, space="PSUM") as cp, \
             tc.tile_pool(name="dp", bufs=1, space="PSUM") as dp, \
             tc.tile_pool(name="pp", bufs=3) as pp, \
             tc.tile_pool(name="ctp", bufs=4) as ctp, \
             tc.tile_pool(name="rdp", bufs=4) as rdp, \
             tc.tile_pool(name="osp", bufs=3) as osp:
            for j in range(3):
                heads = (2 * j, 2 * j + 1)
                ostage = {h: osp.tile([64, SEQ], F32, tag="os", name=f"os{h}")
                          for h in heads}
                for qh in range(2):
                    q0 = qh * 1024
                    cab = cp.tile([128, 1024], F32, tag="c", name="cab")
                    dn = dp.tile([97, 1024], F32, tag="d", name="dn")
                    for kb in range(16):
                        ks = slice(kb * 128, (kb + 1) * 128)
                        sab = [sp.tile([128, 1024], F32, tag="s", name="sab")
                               for _ in range(2)]
                        # scores: 2-head row-packed pairs (K=64 at rows 0/64)
                        for qq in range(2):
                            qs = slice(q0 + qq * 512, q0 + (qq + 1) * 512)
                            osl = slice(qq * 512, (qq + 1) * 512)
                            for i in range(2):
                                rows = slice(64 * i, 64 * (i + 1))
                                nc.tensor.matmul(sab[i][:, osl],
                                                 lhsT=kt[j][rows, ks],
                                                 rhs=qt[j][rows, qs],
                                                 start=True, stop=True)
                        pab = []
                        for i in range(2):
                            p = pp.tile([128, 1024], DT_MM, tag="p", name="ptile")
                            nc.scalar.activation(out=p, in_=sab[i], func=Exp,
                                                 scale=0.125)
                            pab.append(p)
                        st, sp_ = (kb == 0), (kb == 15)
                        # ctx: col-packed pair (head A -> out rows 0-63,
                        # head B -> rows 64-127 of the same PSUM tile)
                        for qq in range(2):
                            osl = slice(qq * 512, (qq + 1) * 512)
                            for i in range(2):
                                nc.tensor.matmul(
                                    cab[64 * i:64 * (i + 1), osl],
                                    lhsT=vt[:, kb, heads[i] * 64:(heads[i] + 1) * 64],
                                    rhs=pab[i][:, osl], start=st, stop=sp_,
                                    skip_group_check=True)
                        # denominators: 4-way col-packed m=1 matmuls
                        # rows 0/32 = heads A/B cols 0:512; 64/96 = cols 512:1024
                        for idx, (i, qq) in enumerate(((0, 0), (1, 0), (0, 1), (1, 1))):
                            osl = slice(qq * 512, (qq + 1) * 512)
                            r = 32 * idx
                            nc.tensor.matmul(dn[r:r + 1, osl],
                                             lhsT=mtile[:, kb:kb + 1],
                                             rhs=pab[i][:, osl],
                                             start=st, stop=sp_,
                                             tile_position=(0, r),
                                             skip_group_check=True)
                    # drain: out = ctx / denom
                    for i in range(2):
                        h = heads[i]
                        ct = ctp.tile([64, 1024], F32, tag="ct")
                        nc.vector.tensor_copy(out=ct, in_=cab[64 * i:64 * (i + 1), :])
                        rd = rdp.tile([1, 1024], DT_MM, tag="rd")
                        nc.vector.tensor_copy(out=rd[:, 0:512],
                                              in_=dn[32 * i:32 * i + 1, 0:512])
                        nc.vector.tensor_copy(out=rd[:, 512:1024],
                                              in_=dn[64 + 32 * i:64 + 32 * i + 1, 512:1024])
                        bc = sp.tile([64, 1024], F32, tag="s")
                        for qq in range(2):
                            osl = slice(qq * 512, (qq + 1) * 512)
                            nc.tensor.matmul(bc[:, osl], lhsT=xt1[:, 0:64],
                                             rhs=rd[:, osl],
                                             start=True, stop=True)
                        rcp = ctp.tile([64, 1024], F32, tag="rcp")
                        nc.vector.reciprocal(rcp, bc)
                        nc.vector.tensor_mul(out=ostage[h][:, q0:q0 + 1024],
                                             in0=ct, in1=rcp)
                for h in heads:
                    nc.sync.dma_start(out=ot_d[h], in_=ostage[h])


def build_nc():
    nc = bacc.Bacc("TRN2")
    xt_d = nc.declare_dram_parameter("xt", [HID + 1, SEQ], DT_MM, isOutput=False)
    wq_d = nc.declare_dram_parameter("wqT", [HID + 1, DSH], DT_MM, isOutput=False)
    wk_d = nc.declare_dram_parameter("wkT", [HID + 1, DSH], DT_MM, isOutput=False)
    wv_d = nc.declare_dram_parameter("wvT", [HID + 1, DSH], DT_MM, isOutput=False)
    mt_d = nc.declare_dram_parameter("mt", [128, 16], DT_MM, isOutput=False)
    ot_d = nc.declare_dram_parameter("OT", [HPC, HD, SEQ], F32, isOutput=True)
    with tile.TileContext(nc) as tc:
        _body(tc, xt_d, wq_d, wk_d, wv_d, mt_d, ot_d)
    nc.finalize()
    return nc


_NC_CACHE = None


def _get_nc():
    global _NC_CACHE
    if _NC_CACHE is None:
        _NC_CACHE = build_nc()
    return _NC_CACHE


def make_in_maps(hidden_states, attention_mask, Wq, bq, Wk, bk, Wv, bv):
    in_maps = []
    for c in range(NCORES):
        b, g = c // 2, c % 2
        hs = slice(g * DSH, (g + 1) * DSH)
        xt = np.empty((HID + 1, SEQ), DT_NP)
        xt[:HID] = hidden_states[b].T
        xt[HID] = 1.0
        m = (attention_mask[b, 0, 0] > -1).astype(DT_NP)
        mt = np.ascontiguousarray(m.reshape(16, 128).T)

        def aug(W, bias):
            wa = np.empty((HID + 1, DSH), DT_NP)
            wa[:HID] = W[hs, :].T
            wa[HID] = bias[hs]
            return wa

        in_maps.append({
            "xt": np.ascontiguousarray(xt),
            "wqT": aug(Wq, bq),
            "wkT": aug(Wk, bk),
            "wvT": aug(Wv, bv),
            "mt": mt,
        })
    return in_maps


def gather_out(results):
    out = np.empty((BS, SEQ, HID), np.float32)
    for c in range(NCORES):
        b, g = c // 2, c % 2
        ot = results[c]["OT"]  # [6, 64, 2048]
        out[b, :, g * DSH:(g + 1) * DSH] = (
            ot.transpose(2, 0, 1).reshape(SEQ, DSH)
        )
    return out


def kernel(hidden_states, attention_mask, Wq, bq, Wk, bk, Wv, bv):
    nc = _get_nc()
    in_maps = make_in_maps(hidden_states, attention_mask,
                           Wq, bq, Wk, bk, Wv, bv)
    res = run_bass_kernel_spmd(nc, in_maps, core_ids=list(range(NCORES)))
    return gather_out(res.results)



# revision 4
# speedup vs baseline: 1.5143x; 1.5143x over previous
"""BERT self-attention (BS=4, SEQ=2048, HID=768, NH=12) on 8 NeuronCores.

Sharding: core c -> batch b = c//2, head-group g = c%2 (6 heads each).

Per core, for its batch element and 6 heads (3 pairs j=0..2):
  Q^T/K^T[j] = W_j @ X^T + b  in [d, q] layout (d on partitions), bias
               folded on the DVE during the PSUM->SBUF copy.
  V          = X @ Wv^T + bv  in [k, d] layout, rows scaled by mask m[k],
               projected inline during the first attention loop.
  S^T        = K^T.T-free matmul -> [k_block, q] scores in PSUM
               (two heads row-tiled at PE rows 0-63 / 64-127).
  P^T        = exp(S^T / 8)   (ACT, PSUM->SBUF f16; mask folded into V)
  ctx^T      = V'.T @ P^T accumulated over k blocks (heads col-tiled into
               rows 0-63 / 64-127 of one PSUM tile).
  denom      = mask-column m=1 matmuls, 4-way col-tiled, into a 1-bank
               PSUM tile.
  out        = ctx^T * broadcast(1/denom), f16, DMA'd per q-half.

The emission is software-pipelined: scores/exp for k-block t are emitted
before ctx/denom for t-1, so the ACT engine (the exp throughput bound)
never waits on the PE and vice versa.  QK projections for head-pair j+1
are interleaved into attention loop j.

Host does input transposes (free), sharding, and the final [d,q]->[q,d]
untranspose + concat.
"""

import numpy as np

import concourse.bass as bass
import concourse.tile as tile
from concourse import bacc
from concourse import mybir
from concourse.bass_utils import run_bass_kernel_spmd

F32 = mybir.dt.float32
F16 = mybir.dt.float16
DT_MM = F16          # dtype for matmul operands
DT_NP = np.float16   # matching numpy dtype for host-side input prep

BS, SEQ, HID, NH, HD = 4, 2048, 768, 12, 64
NCORES = 8
HPC = 6          # heads per core
FCH = 6          # 128-row chunks of the 768 contraction dim
DSH = HPC * HD   # 384 output features per core


def _body(tc, xt_d, wq_d, wk_d, wv_d, bqc_d, mt_d, ot_d):
    nc = tc.nc
    Exp = mybir.ActivationFunctionType.Exp

    with tc.tile_pool(name="persist", bufs=1) as P, \
         tc.tile_pool(name="work", bufs=1) as W, \
         tc.tile_pool(name="ps", bufs=1, space="PSUM") as PS:
        # Warm the exp table set ASAP (overlaps the input DMAs).
        dummy = P.tile([1, 1], F32, tag="dummy")
        nc.vector.memset(dummy, 0.0)
        nc.scalar.activation(out=dummy, in_=dummy, func=Exp)

        # ---------------- input DMAs ----------------
        xts = []
        for f in range(FCH):
            t = P.tile([128, SEQ], DT_MM, tag=f"x{f}", name=f"x{f}")
            nc.sync.dma_start(out=t, in_=xt_d[f * 128:(f + 1) * 128, :])
            xts.append(t)

        wmap = {}
        for dram, nm in ((wq_d, "q"), (wk_d, "k")):
            lst = []
            for f in range(FCH):
                t = P.tile([128, DSH], DT_MM, tag=f"w{nm}{f}", name=f"w{nm}{f}")
                nc.sync.dma_start(out=t, in_=dram[f * 128:(f + 1) * 128, :])
                lst.append(t)
            wmap[nm] = lst

        bqc = P.tile([128, 6], F32, tag="bqc")
        nc.sync.dma_start(out=bqc, in_=bqc_d[:, :])

        mtile = P.tile([128, 16], DT_MM, tag="mtile")
        nc.sync.dma_start(out=mtile, in_=mt_d[:, :])
        mtf = P.tile([128, 16], F32, tag="mtf")
        nc.vector.tensor_copy(out=mtf, in_=mtile)

        xt1 = P.tile([1, SEQ], DT_MM, tag="x6")
        nc.sync.dma_start(out=xt1, in_=xt_d[768:769, :])

        wv = []
        for f in range(FCH):
            t = P.tile([128, DSH], DT_MM, tag=f"wv{f}", name=f"wv{f}")
            nc.sync.dma_start(out=t, in_=wv_d[f * 128:(f + 1) * 128, :])
            wv.append(t)
        wvb = P.tile([1, DSH], DT_MM, tag="wvb")
        nc.sync.dma_start(out=wvb, in_=wv_d[768:769, :])

        # ---------------- persistent compute tiles ----------------
        qt = [P.tile([128, SEQ], DT_MM, tag=f"qt{j}", name=f"qt{j}") for j in range(3)]
        kt = [P.tile([128, SEQ], DT_MM, tag=f"kt{j}", name=f"kt{j}") for j in range(3)]
        vt = P.tile([128, 16, DSH], DT_MM, tag="vt")

        def proj_chunk(nm, j, qc, tag, bufs=1):
            """One [128,512] chunk of Q^T[j] or K^T[j]: 6 accumulating
            matmuls + a DVE copy that folds the bias (per-partition)."""
            qs = slice(qc * 512, (qc + 1) * 512)
            ps = PS.tile([128, 512], F32, tag=tag, name=f"pj{nm}{j}{qc}",
                         bufs=bufs)
            wt = wmap[nm]
            js = slice(j * 128, (j + 1) * 128)
            for f in range(FCH):
                nc.tensor.matmul(ps, lhsT=wt[f][:, js], rhs=xts[f][:, qs],
                                 start=(f == 0), stop=(f == FCH - 1))
            dst = qt[j] if nm == "q" else kt[j]
            bcol = j if nm == "q" else 3 + j
            nc.vector.tensor_scalar_add(out=dst[:, qs], in0=ps,
                                        scalar1=bqc[:, bcol:bcol + 1])

        def v_chunk(kb):
            """V rows for k-block kb, all 6 heads, mask-scaled into vt."""
            ks = slice(kb * 128, (kb + 1) * 128)
            vps = PS.tile([128, DSH], F32, tag="x", name=f"vps{kb}")
            for f in range(FCH):
                nc.tensor.matmul(vps, lhsT=xts[f][:, ks], rhs=wv[f],
                                 start=(f == 0), stop=False)
            nc.tensor.matmul(vps, lhsT=xt1[:, ks], rhs=wvb,
                             start=False, stop=True)
            nc.vector.tensor_scalar_mul(out=vt[:, kb, :], in0=vps,
                                        scalar1=mtf[:, kb:kb + 1])

        # Lead-in: project Q^T[0], K^T[0] using the (idle) scores buffers
        # so the chunks double-buffer.
        for nm in ("q", "k"):
            for qc in range(4):
                proj_chunk(nm, 0, qc, "s", bufs=2)

        # ---------------- attention ----------------
        # PSUM tags: s 2x[128,1024] (4 banks) + c [128,1024] (2 banks) +
        # d [97,512] (1 bank) + x [128,512] (1 bank) = 8 banks.
        for j in range(3):
            heads = (2 * j, 2 * j + 1)
            # projection chunks for the next head pair, spread over slots
            chunks = []
            if j < 2:
                chunks = [(nm, j + 1, qc) for nm in ("q", "k") for qc in range(4)]
            for qh in range(2):
                q0 = qh * 1024
                cab = PS.tile([128, 1024], F32, tag="c", name="cab")
                dnt = PS.tile([97, 512], F32, tag="d", name="dnt")
                prev = None
                for kb in range(16):
                    ks = slice(kb * 128, (kb + 1) * 128)
                    # scores for kb (row-tiled head pairs, ping-pong bufs)
                    sab = [PS.tile([128, 1024], F32, tag="s", name="sab", bufs=2)
                           for _ in range(2)]
                    for qq in range(2):
                        qs = slice(q0 + qq * 512, q0 + (qq + 1) * 512)
                        osl = slice(qq * 512, (qq + 1) * 512)
                        for i in range(2):
                            rows = slice(64 * i, 64 * (i + 1))
                            nc.tensor.matmul(sab[i][:, osl],
                                             lhsT=kt[j][rows, ks],
                                             rhs=qt[j][rows, qs],
                                             start=True, stop=True)
                    # exp for kb
                    pab = [W.tile([128, 1024], DT_MM, tag="p", name="ptile", bufs=6)
                           for _ in range(2)]
                    for i in range(2):
                        nc.scalar.activation(out=pab[i], in_=sab[i], func=Exp,
                                             scale=0.125)
                    # V projection for kb (consumed by ctx at slot kb+1)
                    if j == 0 and qh == 0:
                        v_chunk(kb)
                    # ctx + denominators for kb-1 (half-iteration lag)
                    if prev is not None:
                        ppab, pkb = prev
                        st, sp_ = (pkb == 0), (pkb == 15)
                        for qq in range(2):
                            osl = slice(qq * 512, (qq + 1) * 512)
                            for i in range(2):
                                nc.tensor.matmul(
                                    cab[64 * i:64 * (i + 1), osl],
                                    lhsT=vt[:, pkb, heads[i] * 64:(heads[i] + 1) * 64],
                                    rhs=ppab[i][:, osl], start=st, stop=sp_,
                                    skip_group_check=True)
                        for idx, (i, qq) in enumerate(((0, 0), (1, 0), (0, 1), (1, 1))):
                            osl = slice(qq * 512, (qq + 1) * 512)
                            r = 32 * idx
                            nc.tensor.matmul(dnt[r:r + 1, :],
                                             lhsT=mtile[:, pkb:pkb + 1],
                                             rhs=ppab[i][:, osl],
                                             start=st, stop=sp_,
                                             tile_position=(0, r),
                                             skip_group_check=True)
                    prev = (pab, kb)
                    # interleaved projection work for j+1
                    if chunks and ((j == 0 and qh == 1 and kb % 2 == 0) or
                                   (j == 1 and qh == 0 and kb % 2 == 0)):
                        nm, pj, qc = chunks.pop(0)
                        proj_chunk(nm, pj, qc, "x")
                # flush ctx/dn for kb=15
                ppab, pkb = prev
                for qq in range(2):
                    osl = slice(qq * 512, (qq + 1) * 512)
                    for i in range(2):
                        nc.tensor.matmul(
                            cab[64 * i:64 * (i + 1), osl],
                            lhsT=vt[:, pkb, heads[i] * 64:(heads[i] + 1) * 64],
                            rhs=ppab[i][:, osl], start=False, stop=True,
                            skip_group_check=True)
                for idx, (i, qq) in enumerate(((0, 0), (1, 0), (0, 1), (1, 1))):
                    osl = slice(qq * 512, (qq + 1) * 512)
                    r = 32 * idx
                    nc.tensor.matmul(dnt[r:r + 1, :],
                                     lhsT=mtile[:, pkb:pkb + 1],
                                     rhs=ppab[i][:, osl],
                                     start=False, stop=True,
                                     tile_position=(0, r),
                                     skip_group_check=True)

                # drain: out = ctx / denom
                cts = []
                for i in range(2):
                    ct = W.tile([64, 1024], F32, tag="ct", name="ct", bufs=4)
                    nc.vector.tensor_copy(out=ct, in_=cab[64 * i:64 * (i + 1), :])
                    cts.append(ct)
                for i in range(2):
                    h = heads[i]
                    rdf = W.tile([1, 1024], F32, tag="rd", name="rdf", bufs=2)
                    nc.vector.tensor_copy(out=rdf[:, 0:512],
                                          in_=dnt[32 * i:32 * i + 1, :])
                    nc.vector.tensor_copy(out=rdf[:, 512:1024],
                                          in_=dnt[64 + 32 * i:64 + 32 * i + 1, :])
                    rcp = W.tile([1, 1024], F32, tag="rcp", name="rcp", bufs=2)
                    nc.vector.reciprocal_approx_fast(out=rcp, in_=rdf)
                    rcp16 = W.tile([1, 1024], DT_MM, tag="rcp16", name="rcp16", bufs=2)
                    nc.vector.tensor_copy(out=rcp16, in_=rcp)
                    ost = W.tile([64, 1024], DT_MM, tag="os", name="ost", bufs=4)
                    for half in range(2):
                        osl = slice(half * 512, (half + 1) * 512)
                        bc = PS.tile([64, 512], F32, tag="x", name="bc")
                        nc.tensor.matmul(bc, lhsT=xt1[:, 0:64],
                                         rhs=rcp16[:, osl],
                                         start=True, stop=True)
                        nc.vector.tensor_mul(out=ost[:, osl],
                                             in0=cts[i][:, osl], in1=bc)
                    nc.sync.dma_start(out=ot_d[h][:, q0:q0 + 1024], in_=ost)


def build_nc():
    nc = bacc.Bacc("TRN2")
    xt_d = nc.declare_dram_parameter("xt", [HID + 1, SEQ], DT_MM, isOutput=False)
    wq_d = nc.declare_dram_parameter("wqT", [HID, DSH], DT_MM, isOutput=False)
    wk_d = nc.declare_dram_parameter("wkT", [HID, DSH], DT_MM, isOutput=False)
    wv_d = nc.declare_dram_parameter("wvT", [HID + 1, DSH], DT_MM, isOutput=False)
    bqc_d = nc.declare_dram_parameter("bqc", [128, 6], F32, isOutput=False)
    mt_d = nc.declare_dram_parameter("mt", [128, 16], DT_MM, isOutput=False)
    ot_d = nc.declare_dram_parameter("OT", [HPC, HD, SEQ], DT_MM, isOutput=True)
    with tile.TileContext(nc) as tc:
        _body(tc, xt_d, wq_d, wk_d, wv_d, bqc_d, mt_d, ot_d)
    nc.finalize()
    return nc


_NC_CACHE = None


def _get_nc():
    global _NC_CACHE
    if _NC_CACHE is None:
        _NC_CACHE = build_nc()
    return _NC_CACHE


def make_in_maps(hidden_states, attention_mask, Wq, bq, Wk, bk, Wv, bv):
    in_maps = []
    for c in range(NCORES):
        b, g = c // 2, c % 2
        hs = slice(g * DSH, (g + 1) * DSH)
        xt = np.empty((HID + 1, SEQ), DT_NP)
        xt[:HID] = hidden_states[b].T
        xt[HID] = 1.0
        m = (attention_mask[b, 0, 0] > -1).astype(DT_NP)
        mt = np.ascontiguousarray(m.reshape(16, 128).T)

        wva = np.empty((HID + 1, DSH), DT_NP)
        wva[:HID] = Wv[hs, :].T
        wva[HID] = bv[hs]

        bqc = np.empty((128, 6), np.float32)
        bqc[:, 0:3] = bq[hs].reshape(3, 128).T
        bqc[:, 3:6] = bk[hs].reshape(3, 128).T

        in_maps.append({
            "xt": np.ascontiguousarray(xt),
            "wqT": np.ascontiguousarray(Wq[hs, :].T.astype(DT_NP)),
            "wkT": np.ascontiguousarray(Wk[hs, :].T.astype(DT_NP)),
            "wvT": wva,
            "bqc": bqc,
            "mt": mt,
        })
    return in_maps


def gather_out(results):
    out = np.empty((BS, SEQ, HID), np.float32)
    for c in range(NCORES):
        b, g = c // 2, c % 2
        ot = results[c]["OT"]  # [6, 64, 2048] f16
        out[b, :, g * DSH:(g + 1) * DSH] = (
            ot.transpose(2, 0, 1).reshape(SEQ, DSH).astype(np.float32)
        )
    return out


def kernel(hidden_states, attention_mask, Wq, bq, Wk, bk, Wv, bv):
    nc = _get_nc()
    in_maps = make_in_maps(hidden_states, attention_mask,
                           Wq, bq, Wk, bk, Wv, bv)
    res = run_bass_kernel_spmd(nc, in_maps, core_ids=list(range(NCORES)))
    return gather_out(res.results)


# revision 9
# speedup vs baseline: 1.7290x; 1.1417x over previous
"""BERT self-attention (BS=4, SEQ=2048, HID=768, NH=12) on 8 NeuronCores.

Sharding: core c -> batch b = c//2, head-group g = c%2 (6 heads each).

Per core, for its batch element and 6 heads (3 pairs j=0..2):
  Q^T/K^T[j] = W_j @ X^T + b  in [d, q] layout (d on partitions), bias
               folded on the DVE during the PSUM->SBUF copy.
  V          = X @ Wv^T + bv  in [k, d] layout, rows scaled by mask m[k],
               projected inline during the first attention loop.
  S^T        = K^T.T-free matmul -> [k_block, q] scores in PSUM
               (two heads row-tiled at PE rows 0-63 / 64-127).
  P^T        = exp(S^T / 8)   (ACT, PSUM->SBUF f16; mask folded into V)
  ctx^T      = V'.T @ P^T accumulated over k blocks (heads col-tiled into
               rows 0-63 / 64-127 of one PSUM tile).
  denom      = mask-column m=1 matmuls, 4-way col-tiled, into a 1-bank
               PSUM tile.
  out        = ctx^T * broadcast(1/denom), f16, DMA'd per q-half.

The emission is software-pipelined: scores/exp for k-block t are emitted
before ctx/denom for t-1, so the ACT engine (the exp throughput bound)
never waits on the PE and vice versa.  QK projections for head-pair j+1
are interleaved into attention loop j.

Host does input transposes (free), sharding, and the final [d,q]->[q,d]
untranspose + concat.
"""

import numpy as np

import concourse.bass as bass
import concourse.tile as tile
from concourse import bacc
from concourse import mybir
from concourse.bass_utils import run_bass_kernel_spmd

F32 = mybir.dt.float32
F16 = mybir.dt.float16
DT_MM = F16          # dtype for matmul operands
DT_NP = np.float16   # matching numpy dtype for host-side input prep

BS, SEQ, HID, NH, HD = 4, 2048, 768, 12, 64
NCORES = 8
HPC = 6          # heads per core
FCH = 6          # 128-row chunks of the 768 contraction dim
DSH = HPC * HD   # 384 output features per core


def _body(tc, xt_d, wq_d, wk_d, wv_d, bqc_d, mt_d, ot_d):
    nc = tc.nc
    Exp = mybir.ActivationFunctionType.Exp

    with tc.tile_pool(name="persist", bufs=1) as P, \
         tc.tile_pool(name="work", bufs=1) as W, \
         tc.tile_pool(name="ps", bufs=1, space="PSUM") as PS:
        # Warm the exp table set ASAP (overlaps the input DMAs).
        dummy = P.tile([1, 1], F32, tag="dummy")
        nc.vector.memset(dummy, 0.0)
        nc.scalar.activation(out=dummy, in_=dummy, func=Exp)

        # ---------------- input DMAs ----------------
        xts = []
        for f in range(FCH):
            t = P.tile([128, SEQ], DT_MM, tag=f"x{f}", name=f"x{f}")
            nc.sync.dma_start(out=t, in_=xt_d[f * 128:(f + 1) * 128, :])
            xts.append(t)

        wmap = {}
        for dram, nm in ((wq_d, "q"), (wk_d, "k")):
            lst = []
            for f in range(FCH):
                t = P.tile([128, DSH], DT_MM, tag=f"w{nm}{f}", name=f"w{nm}{f}")
                nc.sync.dma_start(out=t, in_=dram[f * 128:(f + 1) * 128, :])
                lst.append(t)
            wmap[nm] = lst

        bqc = P.tile([128, 6], F32, tag="bqc")
        nc.sync.dma_start(out=bqc, in_=bqc_d[:, :])

        mtile = P.tile([128, 16], DT_MM, tag="mtile")
        nc.sync.dma_start(out=mtile, in_=mt_d[:, :])
        mtf = P.tile([128, 16], F32, tag="mtf")
        nc.vector.tensor_copy(out=mtf, in_=mtile)

        xt1 = P.tile([1, SEQ], DT_MM, tag="x6")
        nc.sync.dma_start(out=xt1, in_=xt_d[768:769, :])

        wv = []
        for f in range(FCH):
            t = P.tile([128, DSH], DT_MM, tag=f"wv{f}", name=f"wv{f}")
            nc.sync.dma_start(out=t, in_=wv_d[f * 128:(f + 1) * 128, :])
            wv.append(t)
        wvb = P.tile([1, DSH], DT_MM, tag="wvb")
        nc.sync.dma_start(out=wvb, in_=wv_d[768:769, :])

        # ---------------- persistent compute tiles ----------------
        qt = [P.tile([128, SEQ], DT_MM, tag=f"qt{j}", name=f"qt{j}") for j in range(3)]
        kt = [P.tile([128, SEQ], DT_MM, tag=f"kt{j}", name=f"kt{j}") for j in range(3)]
        vt = P.tile([128, 16, DSH], DT_MM, tag="vt")

        def proj_chunk(nm, j, qc, tag, bufs=1):
            """One [128,512] chunk of Q^T[j] or K^T[j]: 6 accumulating
            matmuls + a DVE copy that folds the bias (per-partition)."""
            qs = slice(qc * 512, (qc + 1) * 512)
            ps = PS.tile([128, 512], F32, tag=tag, name=f"pj{nm}{j}{qc}",
                         bufs=bufs)
            wt = wmap[nm]
            js = slice(j * 128, (j + 1) * 128)
            for f in range(FCH):
                nc.tensor.matmul(ps, lhsT=wt[f][:, js], rhs=xts[f][:, qs],
                                 start=(f == 0), stop=(f == FCH - 1))
            dst = qt[j] if nm == "q" else kt[j]
            bcol = j if nm == "q" else 3 + j
            nc.vector.tensor_scalar_add(out=dst[:, qs], in0=ps,
                                        scalar1=bqc[:, bcol:bcol + 1])

        def v_chunk(kb):
            """V rows for k-block kb, all 6 heads, mask-scaled into vt."""
            ks = slice(kb * 128, (kb + 1) * 128)
            vps = PS.tile([128, DSH], F32, tag="x", name=f"vps{kb}")
            for f in range(FCH):
                nc.tensor.matmul(vps, lhsT=xts[f][:, ks], rhs=wv[f],
                                 start=(f == 0), stop=False)
            nc.tensor.matmul(vps, lhsT=xt1[:, ks], rhs=wvb,
                             start=False, stop=True)
            nc.vector.tensor_scalar_mul(out=vt[:, kb, :], in0=vps,
                                        scalar1=mtf[:, kb:kb + 1])

        # PE warmup: dense dummy matmuls during the input DMA wait so the
        # HAM clock gate opens (1.2 -> 2.4 GHz) before real work starts.
        warm = P.tile([128, 512], DT_MM, tag="warm")
        nc.vector.memset(warm, 0.0)
        for _ in range(14):
            wps = PS.tile([128, 512], F32, tag="x", name="wps")
            nc.tensor.matmul(wps, lhsT=warm[:, 0:128], rhs=warm,
                             start=True, stop=True)

        # Lead-in: project Q^T[0], K^T[0] using the (idle) scores buffers
        # so the chunks double-buffer.
        for nm in ("q", "k"):
            for qc in range(4):
                proj_chunk(nm, 0, qc, "s", bufs=2)

        # ---------------- attention ----------------
        # PSUM tags: s 2x[128,1024] (4 banks) + c [128,1024] (2 banks) +
        # d [97,512] (1 bank) + x [128,512] (1 bank) = 8 banks.
        pending = []   # deferred bc/mul/dma work from the previous drain
        for j in range(3):
            heads = (2 * j, 2 * j + 1)
            # projection chunks for the next head pair, spread over slots
            chunks = []
            if j < 2:
                chunks = [(nm, j + 1, qc) for nm in ("q", "k") for qc in range(4)]
            for qh in range(2):
                q0 = qh * 1024
                cab = PS.tile([128, 1024], F32, tag="c", name="cab")
                dnt = PS.tile([97, 512], F32, tag="d", name="dnt")
                prev = None
                for kb in range(16):
                    ks = slice(kb * 128, (kb + 1) * 128)
                    # scores for kb (row-tiled head pairs, ping-pong bufs)
                    sab = [PS.tile([128, 1024], F32, tag="s", name="sab", bufs=2)
                           for _ in range(2)]
                    for qq in range(2):
                        qs = slice(q0 + qq * 512, q0 + (qq + 1) * 512)
                        osl = slice(qq * 512, (qq + 1) * 512)
                        for i in range(2):
                            rows = slice(64 * i, 64 * (i + 1))
                            nc.tensor.matmul(sab[i][:, osl],
                                             lhsT=kt[j][rows, ks],
                                             rhs=qt[j][rows, qs],
                                             start=True, stop=True)
                    # exp for kb
                    pab = [W.tile([128, 1024], DT_MM, tag="p", name="ptile", bufs=6)
                           for _ in range(2)]
                    for i in range(2):
                        nc.scalar.activation(out=pab[i], in_=sab[i], func=Exp,
                                             scale=0.125)
                    # V projection for kb (consumed by ctx at slot kb+1)
                    if j == 0 and qh == 0:
                        v_chunk(kb)
                    # ctx + denominators for kb-1 (half-iteration lag)
                    if prev is not None:
                        ppab, pkb = prev
                        st, sp_ = (pkb == 0), (pkb == 15)
                        for qq in range(2):
                            osl = slice(qq * 512, (qq + 1) * 512)
                            for i in range(2):
                                nc.tensor.matmul(
                                    cab[64 * i:64 * (i + 1), osl],
                                    lhsT=vt[:, pkb, heads[i] * 64:(heads[i] + 1) * 64],
                                    rhs=ppab[i][:, osl], start=st, stop=sp_,
                                    skip_group_check=True)
                        for idx, (i, qq) in enumerate(((0, 0), (1, 0), (0, 1), (1, 1))):
                            osl = slice(qq * 512, (qq + 1) * 512)
                            r = 32 * idx
                            nc.tensor.matmul(dnt[r:r + 1, :],
                                             lhsT=mtile[:, pkb:pkb + 1],
                                             rhs=ppab[i][:, osl],
                                             start=st, stop=sp_,
                                             tile_position=(0, r),
                                             skip_group_check=True)
                    prev = (pab, kb)
                    # deferred normalize/store work from the previous drain
                    if pending and 2 <= kb <= 5:
                        pending.pop(0)()
                    # interleaved projection work for j+1
                    if chunks and 6 <= kb <= 13 and ((j == 0 and qh == 1) or
                                                     (j == 1 and kb % 2 == 0)):
                        nm, pj, qc = chunks.pop(0)
                        proj_chunk(nm, pj, qc, "x")
                # flush ctx/dn for kb=15
                ppab, pkb = prev
                for qq in range(2):
                    osl = slice(qq * 512, (qq + 1) * 512)
                    for i in range(2):
                        nc.tensor.matmul(
                            cab[64 * i:64 * (i + 1), osl],
                            lhsT=vt[:, pkb, heads[i] * 64:(heads[i] + 1) * 64],
                            rhs=ppab[i][:, osl], start=False, stop=True,
                            skip_group_check=True)
                for idx, (i, qq) in enumerate(((0, 0), (1, 0), (0, 1), (1, 1))):
                    osl = slice(qq * 512, (qq + 1) * 512)
                    r = 32 * idx
                    nc.tensor.matmul(dnt[r:r + 1, :],
                                     lhsT=mtile[:, pkb:pkb + 1],
                                     rhs=ppab[i][:, osl],
                                     start=False, stop=True,
                                     tile_position=(0, r),
                                     skip_group_check=True)

                # drain (DVE only): free cab/dnt quickly, compute 1/denom.
                # The bc-broadcast matmuls + final multiplies + store DMAs
                # are deferred into slots 2-5 of the next loop so no
                # long-waiting matmul ever clogs the PE wait queue at the
                # loop boundary.
                cts, r16s = [], []
                for i in range(2):
                    ct = W.tile([64, 1024], F32, tag="ct", name="ct", bufs=4)
                    nc.vector.tensor_copy(out=ct, in_=cab[64 * i:64 * (i + 1), :])
                    cts.append(ct)
                for i in range(2):
                    rdf = W.tile([1, 1024], F32, tag="rd", name="rdf", bufs=2)
                    nc.vector.tensor_copy(out=rdf[:, 0:512],
                                          in_=dnt[32 * i:32 * i + 1, :])
                    nc.vector.tensor_copy(out=rdf[:, 512:1024],
                                          in_=dnt[64 + 32 * i:64 + 32 * i + 1, :])
                    rcp = W.tile([1, 1024], F32, tag="rcp", name="rcp", bufs=2)
                    nc.vector.reciprocal_approx_fast(out=rcp, in_=rdf)
                    rcp16 = W.tile([1, 1024], DT_MM, tag="rcp16", name="rcp16", bufs=2)
                    nc.vector.tensor_copy(out=rcp16, in_=rcp)
                    r16s.append(rcp16)
                osts = [W.tile([64, 1024], DT_MM, tag="os", name="ost", bufs=4)
                        for _ in range(2)]

                def normalize_store(i, half, q0=q0, heads=heads,
                                    cts=cts, r16s=r16s, osts=osts):
                    osl = slice(half * 512, (half + 1) * 512)
                    bc = PS.tile([64, 512], F32, tag="x", name="bc")
                    nc.tensor.matmul(bc, lhsT=xt1[:, 0:64],
                                     rhs=r16s[i][:, osl],
                                     start=True, stop=True)
                    nc.vector.tensor_mul(out=osts[i][:, osl],
                                         in0=cts[i][:, osl], in1=bc)
                    if half == 1:
                        nc.sync.dma_start(out=ot_d[heads[i]][:, q0:q0 + 1024],
                                          in_=osts[i])

                pending = [lambda i=i, half=half: normalize_store(i, half)
                           for i in range(2) for half in range(2)]
        # tail: final drain work has no following loop
        for fn in pending:
            fn()


def build_nc():
    nc = bacc.Bacc("TRN2")
    xt_d = nc.declare_dram_parameter("xt", [HID + 1, SEQ], DT_MM, isOutput=False)
    wq_d = nc.declare_dram_parameter("wqT", [HID, DSH], DT_MM, isOutput=False)
    wk_d = nc.declare_dram_parameter("wkT", [HID, DSH], DT_MM, isOutput=False)
    wv_d = nc.declare_dram_parameter("wvT", [HID + 1, DSH], DT_MM, isOutput=False)
    bqc_d = nc.declare_dram_parameter("bqc", [128, 6], F32, isOutput=False)
    mt_d = nc.declare_dram_parameter("mt", [128, 16], DT_MM, isOutput=False)
    ot_d = nc.declare_dram_parameter("OT", [HPC, HD, SEQ], DT_MM, isOutput=True)
    with tile.TileContext(nc) as tc:
        _body(tc, xt_d, wq_d, wk_d, wv_d, bqc_d, mt_d, ot_d)
    nc.finalize()
    return nc


_NC_CACHE = None


def _get_nc():
    global _NC_CACHE
    if _NC_CACHE is None:
        _NC_CACHE = build_nc()
    return _NC_CACHE


def make_in_maps(hidden_states, attention_mask, Wq, bq, Wk, bk, Wv, bv):
    in_maps = []
    for c in range(NCORES):
        b, g = c // 2, c % 2
        hs = slice(g * DSH, (g + 1) * DSH)
        xt = np.empty((HID + 1, SEQ), DT_NP)
        xt[:HID] = hidden_states[b].T
        xt[HID] = 1.0
        m = (attention_mask[b, 0, 0] > -1).astype(DT_NP)
        mt = np.ascontiguousarray(m.reshape(16, 128).T)

        wva = np.empty((HID + 1, DSH), DT_NP)
        wva[:HID] = Wv[hs, :].T
        wva[HID] = bv[hs]

        bqc = np.empty((128, 6), np.float32)
        bqc[:, 0:3] = bq[hs].reshape(3, 128).T
        bqc[:, 3:6] = bk[hs].reshape(3, 128).T

        in_maps.append({
            "xt": np.ascontiguousarray(xt),
            "wqT": np.ascontiguousarray(Wq[hs, :].T.astype(DT_NP)),
            "wkT": np.ascontiguousarray(Wk[hs, :].T.astype(DT_NP)),
            "wvT": wva,
            "bqc": bqc,
            "mt": mt,
        })
    return in_maps


def gather_out(results):
    out = np.empty((BS, SEQ, HID), np.float32)
    for c in range(NCORES):
        b, g = c // 2, c % 2
        ot = results[c]["OT"]  # [6, 64, 2048] f16
        out[b, :, g * DSH:(g + 1) * DSH] = (
            ot.transpose(2, 0, 1).reshape(SEQ, DSH).astype(np.float32)
        )
    return out


def kernel(hidden_states, attention_mask, Wq, bq, Wk, bk, Wv, bv):
    nc = _get_nc()
    in_maps = make_in_maps(hidden_states, attention_mask,
                           Wq, bq, Wk, bk, Wv, bv)
    res = run_bass_kernel_spmd(nc, in_maps, core_ids=list(range(NCORES)))
    return gather_out(res.results)
